# revision 1
# baseline (speedup 1.0000x reference)
"""Causal self-attention (GQA + rope + rms-norm + int4 fake-quant weights) on 8 trn2 cores.

Sharding: core = (batch b, kv-group g); b = core // 4, g = core % 4.
Each core computes heads 4g..4g+3 of batch b end-to-end through attention,
AllGathers y.T across its 4-core batch group, and produces the output
projection slice out[b, :, 256g:256g+256] (w_proj row-split keeps the
per-row int4 quantization exact). Host does pure slicing / concat only.

Attention is computed in transposed-score form: scoresT[k, q], so the
softmax denominator comes from an ones-augmented v column via the same
matmul that computes attn@v, and no per-tile transposes of the attention
matrix are needed. Softmax uses no max-subtraction: rms-normalised q, k
bound |score| <= 8*|gain|, so exp() cannot overflow in fp32.
"""

import sys

sys.path.insert(0, "/opt/trn_rl_repo")

import functools
import os
import numpy as np

import jax

jax.config.update("jax_compilation_cache_dir", "/tmp/jax_cache")
jax.config.update("jax_persistent_cache_min_entry_size_bytes", -1)
jax.config.update("jax_persistent_cache_min_compile_time_secs", 0)

import concourse.bass as bass
import concourse.mybir as mybir
import concourse.tile as tile
from concourse import bacc
from concourse.bass_utils import run_bass_kernel_spmd

F32 = mybir.dt.float32
F32R = mybir.dt.float32r
AF = mybir.ActivationFunctionType
ALU = mybir.AluOpType

B, S, D = 2, 2048, 1024
H, KVH, HD = 16, 4, 64
G = 4  # kv head groups (tensor-parallel ways)
N_CORES = 8
P = 128
CH = 512  # seq chunk for matmul free dim
NCH = S // CH  # 4
KT = D // P  # 8 contraction tiles over model dim
QROWS = H // G * HD  # 256 q dims per core
EPS = 1.1920929e-7
MAGIC = 12582912.0  # 1.5*2**23: x + MAGIC - MAGIC == round-half-even(x) for |x| <= 2**22
ROPE_BASE = 10000.0


def build_nc(n_cores=N_CORES, group_size=G, debug=False, phases=9, no_cc=False, repeat=1):
    nc = bacc.Bacc("TRN2", target_bir_lowering=False, debug=False, num_devices=n_cores)
    groups = [list(range(s, s + group_size)) for s in range(0, n_cores, group_size)]

    x_in = nc.dram_tensor("x", [S, D], F32, kind="ExternalInput").ap()
    wq_in = nc.dram_tensor("wq", [QROWS, D], F32, kind="ExternalInput").ap()
    wkv_in = nc.dram_tensor("wkv", [2 * HD, D], F32, kind="ExternalInput").ap()
    wp_in = nc.dram_tensor("wp", [QROWS, D], F32, kind="ExternalInput").ap()
    qgain_in = nc.dram_tensor("qgain", [2, 2], F32, kind="ExternalInput").ap()
    cos2_in = nc.dram_tensor("cos2", [P, S], F32, kind="ExternalInput").ap()
    sin2_in = nc.dram_tensor("sin2", [P, S], F32, kind="ExternalInput").ap()
    ident_in = nc.dram_tensor("ident", [P, P], F32, kind="ExternalInput").ap()
    ones_in = nc.dram_tensor("ones", [P, 1], F32, kind="ExternalInput").ap()
    onesrow_in = nc.dram_tensor("onesrow", [1, HD], F32, kind="ExternalInput").ap()
    bd_in = nc.dram_tensor("bd", [P, 2], F32, kind="ExternalInput").ap()
    bd2_in = nc.dram_tensor("bd2", [2, P], F32, kind="ExternalInput").ap()
    dmask_in = nc.dram_tensor("dmask", [P, P], F32, kind="ExternalInput").ap()
    out = nc.dram_tensor("out", [S, QROWS], F32, kind="ExternalOutput").ap()
    if debug:
        dbg_q = nc.dram_tensor("dbg_q", [HD, S], F32, kind="ExternalOutput").ap()
        dbg_k = nc.dram_tensor("dbg_k", [HD, S], F32, kind="ExternalOutput").ap()
        dbg_v = nc.dram_tensor("dbg_v", [P, (S // P) * (HD + 1)], F32, kind="ExternalOutput").ap()
        dbg_e = nc.dram_tensor("dbg_e", [P, CH], F32, kind="ExternalOutput").ap()
        dbg_y = nc.dram_tensor("dbg_y", [G * QROWS, CH], F32, kind="ExternalOutput").ap()
        dbg_w = nc.dram_tensor("dbg_w", [5 * P, D], F32, kind="ExternalOutput").ap()
        dbg_t = nc.dram_tensor("dbg_t", [P, D], F32, kind="ExternalOutput").ap()
        dbg_sc = nc.dram_tensor("dbg_sc", [P, 3], F32, kind="ExternalOutput").ap()

    with tile.TileContext(nc) as tc:
        with (
            tc.tile_pool(name="consts", bufs=1) as cp,
            tc.tile_pool(name="nat", bufs=2) as natp,
            tc.tile_pool(name="wT", bufs=1) as wtp,
            tc.tile_pool(name="xT", bufs=1) as xtp,
            tc.tile_pool(name="persist", bufs=1) as pp,
            tc.tile_pool(name="work", bufs=2) as wk,
            tc.tile_pool(name="dram", bufs=4, space="DRAM") as dp,
            tc.tile_pool(name="ps_mm", bufs=3, space="PSUM") as ps_mm,
            tc.tile_pool(name="ps_o", bufs=2, space="PSUM") as ps_o,
            tc.tile_pool(name="ps_tr", bufs=1, space="PSUM") as ps_tr,
            tc.tile_pool(name="ps_fp", bufs=1, space="PSUM") as ps_fp,
            tc.tile_pool(name="ps_ssq", bufs=1, space="PSUM") as ps_ssq,
        ):
            rep_ctx = tc.For_i(0, repeat, 1) if repeat > 1 else None
            if rep_ctx is not None:
                rep_ctx.__enter__()
            # ---- constants ----
            ident = cp.tile([P, P], F32R, tag="ident")
            nc.sync.dma_start(ident[:], ident_in[:].bitcast(F32R))
            ones = cp.tile([P, 1], F32R, tag="ones")
            nc.sync.dma_start(ones[:], ones_in[:].bitcast(F32R))
            onesrow = cp.tile([1, HD], F32R, tag="onesrow")
            nc.sync.dma_start(onesrow[:], onesrow_in[:].bitcast(F32R))
            bd = cp.tile([P, 2], F32R, tag="bd")
            nc.sync.dma_start(bd[:], bd_in[:].bitcast(F32R))
            bd2 = cp.tile([2, P], F32R, tag="bd2")
            nc.sync.dma_start(bd2[:], bd2_in[:].bitcast(F32R))
            dmask = cp.tile([P, P], F32R, tag="dmask")
            nc.sync.dma_start(dmask[:], dmask_in[:].bitcast(F32R))
            epsb = cp.tile([2, 1], F32, tag="epsb")
            nc.any.memset(epsb[:], EPS)
            seven = cp.tile([P, 1], F32, tag="seven")
            nc.any.memset(seven[:], 7.0)
            g8 = cp.tile([2, 2], F32, tag="g8")
            nc.sync.dma_start(g8[:], qgain_in[:])
            nc.scalar.mul(g8[:], g8[:], 0.125)

            # ---- weights: quant + transpose ----
            wqT = wtp.tile([P, KT, QROWS], F32R, tag="wqT")
            wkvT = wtp.tile([P, KT, 2 * HD], F32R, tag="wkvT")
            wpT = wtp.tile([P, KT, QROWS], F32R, tag="wpT")
            for src, dstT, nblk in ((wq_in, wqT, 2), (wkv_in, wkvT, 1), (wp_in, wpT, 2)):
                pb = src.shape[0] // nblk
                for blk in range(nblk):
                    w_nat = natp.tile([P, D], F32, tag="w_nat", bufs=2)
                    nc.gpsimd.dma_start(w_nat[:pb], src[blk * pb : (blk + 1) * pb, :])
                    # per-row int4 fake-quant, matching the reference bit-for-bit:
                    # scale = max(rowmax(|w|), 1e-8)/7; round-half-even via 2^23 trick
                    aw = wk.tile([P, D], F32, tag="q_corr", bufs=1)
                    nc.vector.tensor_scalar(aw[:pb], w_nat[:pb], -1.0, None, ALU.mult)
                    nc.vector.tensor_tensor(aw[:pb], aw[:pb], w_nat[:pb], ALU.max)
                    m = wk.tile([P, 1], F32, tag="q_m", bufs=1)
                    nc.vector.tensor_reduce(
                        m[:pb], aw[:pb], axis=mybir.AxisListType.X, op=ALU.max
                    )
                    nc.vector.tensor_scalar(m[:pb], m[:pb], 1e-8, None, ALU.max)
                    # scale = fl(mx/7) exactly: q0 = mx*C17; r = mx - 7*q0 computed
                    # exactly as (mx - 8*q0) + q0 (8*q0 exact, both sums Sterbenz);
                    # s = q0 + r*C17 is then the correctly rounded quotient.
                    C17 = 0.14285714285714285
                    scale = wk.tile([P, 1], F32, tag="q_scale", bufs=1)
                    nc.vector.tensor_scalar(scale[:pb], m[:pb], C17, None, ALU.mult)
                    tq = wk.tile([P, 1], F32, tag="q_tmp", bufs=1)
                    nc.vector.tensor_scalar(tq[:pb], scale[:pb], -8.0, None, ALU.mult)
                    nc.vector.tensor_tensor(tq[:pb], tq[:pb], m[:pb], ALU.add)
                    nc.vector.tensor_tensor(tq[:pb], tq[:pb], scale[:pb], ALU.add)
                    nc.vector.tensor_scalar(tq[:pb], tq[:pb], C17, None, ALU.mult)
                    nc.vector.tensor_tensor(scale[:pb], scale[:pb], tq[:pb], ALU.add)
                    rsc = wk.tile([P, 1], F32, tag="q_rsc", bufs=1)
                    with nc.allow_low_precision(reason="quant reciprocal"):
                        nc.vector.reciprocal(rsc[:pb], scale[:pb])
                    wq_t = wk.tile([P, D], F32, tag="q_wq", bufs=1)
                    nc.scalar.activation(
                        wq_t[:pb], w_nat[:pb], AF.Copy, bias=MAGIC, scale=rsc[:pb]
                    )
                    nc.scalar.activation(
                        wq_t[:pb], wq_t[:pb], AF.Copy, bias=-MAGIC, scale=1.0
                    )
                    if debug and src is wq_in and blk == 0:
                        nc.sync.dma_start(dbg_t[:], w_nat[:])
                        nc.sync.dma_start(dbg_sc[:, 0:1], m[:])
                        nc.sync.dma_start(dbg_sc[:, 1:2], scale[:])
                        nc.sync.dma_start(dbg_sc[:, 2:3], rsc[:])
                    # exact-nearest correction: the reciprocal-based round can be
                    # off by one step near half-integer boundaries; compare the
                    # residual d = n*s - w against +-s/2 and adjust n by +-1.
                    negs = wk.tile([P, 1], F32, tag="q_negs", bufs=1)
                    nc.vector.tensor_scalar(negs[:pb], scale[:pb], -1.0, None, ALU.mult)
                    resid = wk.tile([P, D], F32, tag="q_resid", bufs=1)
                    nc.vector.tensor_scalar_mul(resid[:pb], wq_t[:pb], scale[:pb])
                    nc.vector.tensor_tensor(
                        resid[:pb], resid[:pb], w_nat[:pb], ALU.subtract
                    )
                    corr = wk.tile([P, D], F32, tag="q_corr", bufs=1)
                    nc.vector.tensor_scalar(
                        corr[:pb], resid[:pb], 2.0, negs[:pb], ALU.mult, ALU.is_lt
                    )
                    nc.vector.tensor_tensor(wq_t[:pb], wq_t[:pb], corr[:pb], ALU.add)
                    nc.vector.tensor_scalar(
                        corr[:pb], resid[:pb], 2.0, scale[:pb], ALU.mult, ALU.is_gt
                    )
                    nc.vector.tensor_tensor(
                        wq_t[:pb], wq_t[:pb], corr[:pb], ALU.subtract
                    )
                    nc.vector.tensor_scalar(
                        wq_t[:pb], wq_t[:pb], 7.0, -7.0, ALU.min, ALU.max
                    )
                    wdq_t = wk.tile([P, D], F32R, tag="q_wdq", bufs=1)
                    wdq = wdq_t[:pb]
                    nc.scalar.activation(
                        wdq, wq_t[:pb], AF.Copy, bias=0.0, scale=scale[:pb]
                    )
                    if debug:
                        widx = {id(wq_in): 0, id(wkv_in): 2, id(wp_in): 3}[id(src)]
                        nc.sync.dma_start(
                            dbg_w[(widx + blk) * P : (widx + blk + 1) * P, :],
                            wdq.bitcast(F32),
                        )
                    for k0 in range(0, KT, 4):
                        tp = ps_tr.tile([P, 4 * P], F32R, tag="tr")
                        for q in range(4):
                            nc.tensor.transpose(
                                tp[:, q * P : q * P + pb],
                                wdq[:, (k0 + q) * P : (k0 + q + 1) * P],
                                ident[:pb, :pb],
                            )
                        nc.vector.tensor_copy(
                            dstT[:, k0 : k0 + 4, blk * pb : (blk + 1) * pb],
                            tp[:].rearrange("p (a b) -> p a b", a=4)[:, :, :pb],
                        )

            # ---- x transpose: xT[k] holds x.T rows [128, S] ----
            xT = [xtp.tile([P, S], F32R, tag=f"xT{k}", name=f"xT{k}") for k in range(KT)]
            for st in range(0, S // P, 4):
                xns = []
                for q in range(4):
                    x_nat = natp.tile(
                        [P, D], F32R, tag="x_nat", bufs=4, name=f"xn{st+q}"
                    )
                    nc.sync.dma_start(
                        x_nat[:],
                        x_in[(st + q) * P : (st + q + 1) * P, :].bitcast(F32R),
                    )
                    xns.append(x_nat)
                for k in range(KT):
                    tp = ps_tr.tile([P, 4 * P], F32R, tag="tr")
                    for q in range(4):
                        nc.tensor.transpose(
                            tp[:, q * P : (q + 1) * P],
                            xns[q][:, k * P : (k + 1) * P],
                            ident[:],
                        )
                    nc.vector.tensor_copy(xT[k][:, st * P : (st + 4) * P], tp[:])

            # ---- persistent attention operands ----
            qTr = [pp.tile([HD, S], F32R, tag=f"qTr{h}", name=f"qTr{h}") for h in range(4)]
            kTr = pp.tile([HD, S], F32R, tag="kTr")
            vAug = pp.tile([P, S // P, HD + 1], F32R, tag="vAug")

            def rope_and_scale(raw, fb_ps, cos_t, sin_t, rows, outs):
                """raw [rows, CH] f32 (pre-norm, pre-rope); fb_ps: psum rms*gain
                factor [rows, CH]; outs = [(dst f32r [64, CH], src row)] splits."""
                qsw = wk.tile([P, CH], F32, tag="qsw", bufs=1)
                for r0 in range(0, rows, HD):
                    nc.gpsimd.tensor_copy(qsw[r0 : r0 + 32], raw[r0 + 32 : r0 + 64])
                    nc.gpsimd.tensor_copy(qsw[r0 + 32 : r0 + 64], raw[r0 : r0 + 32])
                t2 = wk.tile([P, CH], F32, tag="t2", bufs=1)
                nc.vector.tensor_mul(t2[:rows], raw[:], cos_t[:rows])
                nc.vector.tensor_mul(qsw[:rows], qsw[:rows], sin_t[:rows])
                nc.vector.tensor_add(qsw[:rows], qsw[:rows], t2[:rows])
                for dst, lo in outs:
                    nc.vector.tensor_mul(dst, qsw[lo : lo + HD], fb_ps[lo : lo + HD])

            # ---- projections (per chunk) ----
            for c in range(NCH if phases >= 1 else 0):
                sl = slice(c * CH, (c + 1) * CH)
                cos_t = wk.tile([P, CH], F32, tag="cos_t", bufs=1)
                nc.gpsimd.dma_start(cos_t[:], cos2_in[:, sl])
                sin_t = wk.tile([P, CH], F32, tag="sin_t", bufs=1)
                nc.gpsimd.dma_start(sin_t[:], sin2_in[:, sl])

                # q: two head pairs
                for mblk in range(2):
                    pq = ps_mm.tile([P, CH], F32, tag="mm")
                    for k in range(KT):
                        nc.tensor.matmul(
                            pq[:],
                            wqT[:, k, mblk * P : (mblk + 1) * P],
                            xT[k][:, sl],
                            start=(k == 0),
                            stop=(k == KT - 1),
                        )
                    q_raw = wk.tile([P, CH], F32, tag="raw", bufs=1)
                    nc.scalar.copy(q_raw[:], pq[:])
                    q2 = wk.tile([P, CH], F32R, tag="sq", bufs=1)
                    nc.scalar.activation(q2[:], pq[:], AF.Square)
                    ssq = ps_ssq.tile([2, CH], F32, tag="ssq")
                    nc.tensor.matmul(ssq[:], bd[:, :], q2[:], start=True, stop=True)
                    srms = wk.tile([2, CH], F32, tag="srms", bufs=1)
                    nc.scalar.activation(
                        srms[:], ssq[:], AF.Sqrt, bias=epsb[:], scale=1.0 / HD
                    )
                    rfac = wk.tile([2, CH], F32R, tag="rfac", bufs=1)
                    with nc.allow_low_precision(reason="f32r matmul feed"):
                        nc.vector.reciprocal(rfac[:], srms[:])
                    nc.vector.tensor_scalar_mul(
                        rfac[:], rfac[:], g8[0:2, mblk : mblk + 1]
                    )
                    fb = ps_mm.tile([P, CH], F32, tag="mm")
                    nc.tensor.matmul(fb[:], bd2[:], rfac[:], start=True, stop=True)
                    rope_and_scale(
                        q_raw[:], fb, cos_t, sin_t, P,
                        [(qTr[2 * mblk][:, sl], 0), (qTr[2 * mblk + 1][:, sl], HD)],
                    )

                # kv
                pkv = ps_mm.tile([P, CH], F32, tag="mm")
                for k in range(KT):
                    nc.tensor.matmul(
                        pkv[:], wkvT[:, k, :], xT[k][:, sl],
                        start=(k == 0), stop=(k == KT - 1),
                    )
                kv_raw = wk.tile([P, CH], F32, tag="raw", bufs=1)
                nc.scalar.copy(kv_raw[:], pkv[:])
                k2 = wk.tile([P, CH], F32R, tag="sq", bufs=1)
                nc.scalar.activation(k2[:HD], pkv[:HD], AF.Square)
                ssk = ps_ssq.tile([2, CH], F32, tag="ssq")
                nc.tensor.matmul(ssk[0:1], ones[:HD], k2[:HD], start=True, stop=True)
                srk = wk.tile([2, CH], F32, tag="srms", bufs=1)
                nc.scalar.activation(
                    srk[0:1], ssk[0:1], AF.Sqrt, bias=epsb[0:1], scale=1.0 / HD
                )
                rfk = wk.tile([2, CH], F32R, tag="rfac", bufs=1)
                with nc.allow_low_precision(reason="f32r matmul feed"):
                    nc.vector.reciprocal(rfk[0:1], srk[0:1])
                fbk = ps_mm.tile([P, CH], F32, tag="mm")
                nc.tensor.matmul(fbk[:HD], onesrow[:], rfk[0:1], start=True, stop=True)
                rope_and_scale(kv_raw[:HD], fbk, cos_t, sin_t, HD, [(kTr[:, sl], 0)])
                # v half -> vAug tiles (s on partitions) + ones column
                v_r = wk.tile([P, CH], F32R, tag="v_r", bufs=1)
                nc.scalar.copy(v_r[:HD], kv_raw[HD:])
                tpv = ps_tr.tile([P, 4 * P], F32R, tag="tr")
                for st in range(CH // P):
                    nc.tensor.transpose(
                        tpv[:, st * P : st * P + HD],
                        v_r[:HD, st * P : (st + 1) * P],
                        ident[:HD, :HD],
                    )
                j0 = c * (CH // P)
                nc.vector.tensor_copy(
                    vAug[:, j0 : j0 + 4, 0:HD],
                    tpv[:].rearrange("p (a b) -> p a b", a=4)[:, :, :HD],
                )
                nc.vector.tensor_copy(
                    vAug[:, j0 : j0 + 4, HD : HD + 1],
                    ones[:, 0:1, None].to_broadcast((P, 4, 1)),
                )

            if debug:
                nc.sync.dma_start(dbg_q[:], qTr[0][:].bitcast(F32))
                nc.sync.dma_start(dbg_k[:], kTr[:].bitcast(F32))
                nc.sync.dma_start(dbg_v[:], vAug[:].bitcast(F32).rearrange("p a b -> p (a b)"))

            # ---- attention + collective + output projection, per chunk ----
            for c in range(NCH if phases >= 2 else 0):
                cc_in = dp.tile([QROWS, CH], F32, tag="cc_in")
                cc_out = dp.tile([G * QROWS, CH], F32, tag="cc_out")
                for h in range(4):
                    po = ps_o.tile([HD + 1, CH], F32, tag="po")
                    njc = 4 * c + 4
                    for j in range(njc):
                        r = j - 4 * c  # >= 0 only on causal-boundary tiles
                        f0 = r * P if r > 0 else 0
                        psc = ps_mm.tile([P, CH], F32, tag="mm")
                        nc.tensor.matmul(
                            psc[:, f0:],
                            kTr[:, j * P : (j + 1) * P],
                            qTr[h][:, c * CH + f0 : (c + 1) * CH],
                            start=True,
                            stop=True,
                            skip_group_check=True,
                        )
                        et = wk.tile([P, CH], F32R, tag="et", bufs=2)
                        nc.scalar.activation(et[:, f0:], psc[:, f0:], AF.Exp)
                        if r >= 0:
                            nc.vector.tensor_mul(
                                et[:, r * P : (r + 1) * P],
                                et[:, r * P : (r + 1) * P],
                                dmask[:],
                            )
                        if debug and c == 0 and h == 0 and j == 0:
                            nc.sync.dma_start(dbg_e[:], et[:].bitcast(F32))
                        nc.tensor.matmul(
                            po[:, f0:],
                            vAug[:, j, :],
                            et[:, f0:],
                            start=(j == 0),
                            stop=(j == njc - 1),
                            skip_group_check=True,
                        )
                    # divide by the softmax sum (row HD of po), broadcast via PE
                    rs = wk.tile([1, CH], F32R, tag="rs", bufs=1)
                    with nc.allow_low_precision(reason="f32r matmul feed"):
                        nc.vector.reciprocal(rs[:], po[HD : HD + 1, :])
                    pr = ps_mm.tile([P, CH], F32, tag="mm")
                    nc.tensor.matmul(pr[:HD], onesrow[:], rs[:], start=True, stop=True)
                    rb = wk.tile([HD, CH], F32, tag="rb", bufs=1)
                    nc.vector.tensor_copy(rb[:], pr[:HD])
                    yt = wk.tile([HD, CH], F32R, tag="yt", bufs=1)
                    nc.vector.tensor_mul(yt[:], po[:HD, :], rb[:])
                    nc.sync.dma_start(cc_in[h * HD : (h + 1) * HD, :], yt[:].bitcast(F32))
                if no_cc:
                    for gg in range(group_size):
                        nc.sync.dma_start(
                            cc_out[gg * QROWS : (gg + 1) * QROWS, :], cc_in[:]
                        )
                else:
                    nc.gpsimd.collective_compute(
                        "AllGather",
                        ALU.bypass,
                        replica_groups=groups,
                        ins=[cc_in.opt()],
                        outs=[cc_out.opt()],
                    )
                if debug and c == 0:
                    nc.sync.dma_start(dbg_y[:], cc_out[:])
                # output projection for this chunk: out[sl] = y_chunk @ wp_g.T
                for stp in range(2):
                    ya = [
                        wk.tile(
                            [P, 2 * P], F32R, tag="ya", bufs=8,
                            name=f"ya{c}_{stp}_{k}",
                        )
                        for k in range(KT)
                    ]
                    for k in range(KT):
                        nc.sync.dma_start(
                            ya[k][:],
                            cc_out[
                                k * P : (k + 1) * P, stp * 2 * P : (stp + 1) * 2 * P
                            ].bitcast(F32R),
                        )
                    for sh in range(2):
                        pf = ps_fp.tile([P, QROWS], F32, tag="fp")
                        for k in range(KT):
                            nc.tensor.matmul(
                                pf[:], ya[k][:, sh * P : (sh + 1) * P], wpT[:, k, :],
                                start=(k == 0), stop=(k == KT - 1),
                            )
                        ot = wk.tile([P, QROWS], F32, tag="ot", bufs=1)
                        nc.scalar.copy(ot[:], pf[:])
                        nc.gpsimd.dma_start(
                            out[
                                c * CH + (stp * 2 + sh) * P : c * CH
                                + (stp * 2 + sh + 1) * P,
                                :,
                            ],
                            ot[:],
                        )
            if rep_ctx is not None:
                rep_ctx.__exit__(None, None, None)

    nc.compile()
    return nc


@functools.lru_cache(maxsize=None)
def get_nc():
    return build_nc()


@functools.lru_cache(maxsize=None)
def host_consts():
    inv_freq = (
        1.0 / (ROPE_BASE ** (np.arange(0, HD, 2, dtype=np.float32) / HD))
    ).astype(np.float32)
    freqs = np.outer(np.arange(S, dtype=np.float32), inv_freq)  # [S, 32]
    cosT = np.cos(freqs).T.astype(np.float32)  # [32, S]
    sinT = np.sin(freqs).T.astype(np.float32)
    cos2 = np.ascontiguousarray(np.tile(cosT, (4, 1)))  # [128, S]
    sin2 = np.ascontiguousarray(
        np.concatenate([sinT, -sinT, sinT, -sinT], axis=0)
    ).astype(np.float32)
    ident = np.eye(P, dtype=np.float32)
    ones = np.ones((P, 1), np.float32)
    onesrow = np.ones((1, HD), np.float32)
    bd = np.zeros((P, 2), np.float32)
    bd[0:HD, 0] = 1.0
    bd[HD:P, 1] = 1.0
    bd2 = np.ascontiguousarray(bd.T)
    # dmask[p, u] = 1 if u >= p (valid region of the causal diagonal tile)
    dmask = (np.arange(P)[None, :] >= np.arange(P)[:, None]).astype(np.float32)
    return dict(
        cos2=cos2, sin2=sin2, ident=ident, ones=ones, onesrow=onesrow,
        bd=bd, bd2=bd2, dmask=dmask,
    )


def make_in_maps(x, w_q, w_k, w_v, w_proj, q_gain, n_cores=N_CORES, group_size=G):
    consts = host_consts()
    in_maps = []
    for core in range(n_cores):
        b, g = core // group_size, core % group_size
        wkv = np.concatenate(
            [w_k[g * HD : (g + 1) * HD, :], w_v[g * HD : (g + 1) * HD, :]], axis=0
        )
        in_maps.append(
            dict(
                x=np.ascontiguousarray(x[b]),
                wq=np.ascontiguousarray(w_q[g * QROWS : (g + 1) * QROWS, :]),
                wkv=np.ascontiguousarray(wkv),
                wp=np.ascontiguousarray(w_proj[g * QROWS : (g + 1) * QROWS, :]),
                qgain=np.ascontiguousarray(q_gain[4 * g : 4 * g + 4].reshape(2, 2).T),
                **consts,
            )
        )
    return in_maps


def assemble(results, n_cores=N_CORES, group_size=G):
    out = np.empty((B, S, D), np.float32)
    for core in range(n_cores):
        b, g = core // group_size, core % group_size
        out[b, :, g * QROWS : (g + 1) * QROWS] = results[core]["out"]
    return out


def kernel(**inputs):
    x = np.asarray(inputs["x"], np.float32)
    w_q = np.asarray(inputs["w_q"], np.float32)
    w_k = np.asarray(inputs["w_k"], np.float32)
    w_v = np.asarray(inputs["w_v"], np.float32)
    w_proj = np.asarray(inputs["w_proj"], np.float32)
    q_gain = np.asarray(inputs["q_gain"], np.float32)

    nc = get_nc()
    in_maps = make_in_maps(x, w_q, w_k, w_v, w_proj, q_gain)
    res = run_bass_kernel_spmd(nc, in_maps, list(range(N_CORES)))
    return assemble(res.results)



# revision 3
# speedup vs baseline: 235.9316x; 235.9316x over previous
"""Causal self-attention (GQA + rope + rms-norm + int4 fake-quant weights) on 8 trn2 cores.

Sharding: core = (batch b, kv-group g); b = core // 4, g = core % 4.
Each core computes heads 4g..4g+3 of batch b end-to-end through attention,
AllGathers y.T across its 4-core batch group, and produces the output
projection slice out[b, :, 256g:256g+256] (w_proj row-split keeps the
per-row int4 quantization exact). Host does slicing / relayout / concat only.

Attention is computed in transposed-score form: scoresT[k, q], so the
softmax denominator comes from an ones-augmented v column via the same
matmul that computes attn@v, and no per-tile transposes of the attention
matrix are needed. Softmax uses no max-subtraction: rms-normalised q, k
bound |score| <= 8*|gain|, so exp() cannot overflow in fp32.

x is shipped pre-transposed per batch (xT = x[b].T, a host relayout), so
the kernel loads x directly in [dim, seq] layout for matmul feeds.
"""

import sys

sys.path.insert(0, "/opt/trn_rl_repo")

import functools
import numpy as np

import jax

jax.config.update("jax_compilation_cache_dir", "/tmp/jax_cache")
jax.config.update("jax_persistent_cache_min_entry_size_bytes", -1)
jax.config.update("jax_persistent_cache_min_compile_time_secs", 0)

import concourse.bass as bass
import concourse.mybir as mybir
import concourse.tile as tile
from concourse import bacc
from concourse.bass_utils import run_bass_kernel_spmd

F32 = mybir.dt.float32
F32R = mybir.dt.float32r
AF = mybir.ActivationFunctionType
ALU = mybir.AluOpType

B, S, D = 2, 2048, 1024
H, KVH, HD = 16, 4, 64
G = 4  # kv head groups (tensor-parallel ways)
N_CORES = 8
P = 128
CH = 512  # seq chunk for matmul free dim
NCH = S // CH  # 4
KT = D // P  # 8 contraction tiles over model dim
QROWS = H // G * HD  # 256 q dims per core
EPS = 1.1920929e-7
MAGIC = 12582912.0  # 1.5*2**23: x + MAGIC - MAGIC == round-half-even(x) for |x| <= 2**22
ROPE_BASE = 10000.0


def build_nc(n_cores=N_CORES, group_size=G, phases=9, no_cc=False, repeat=1):
    nc = bacc.Bacc("TRN2", target_bir_lowering=False, debug=False, num_devices=n_cores)
    groups = [list(range(s, s + group_size)) for s in range(0, n_cores, group_size)]

    xT_in = nc.dram_tensor("xT", [D, S], F32, kind="ExternalInput").ap()
    wq_in = nc.dram_tensor("wq", [QROWS, D], F32, kind="ExternalInput").ap()
    wkv_in = nc.dram_tensor("wkv", [2 * HD, D], F32, kind="ExternalInput").ap()
    wp_in = nc.dram_tensor("wp", [QROWS, D], F32, kind="ExternalInput").ap()
    qgain_in = nc.dram_tensor("qgain", [2, 2], F32, kind="ExternalInput").ap()
    cos2_in = nc.dram_tensor("cos2", [P, S], F32, kind="ExternalInput").ap()
    sin2_in = nc.dram_tensor("sin2", [P, S], F32, kind="ExternalInput").ap()
    ident_in = nc.dram_tensor("ident", [P, P], F32, kind="ExternalInput").ap()
    ones_in = nc.dram_tensor("ones", [P, 1], F32, kind="ExternalInput").ap()
    onesrow_in = nc.dram_tensor("onesrow", [1, HD], F32, kind="ExternalInput").ap()
    bd_in = nc.dram_tensor("bd", [P, 2], F32, kind="ExternalInput").ap()
    bd2_in = nc.dram_tensor("bd2", [2, P], F32, kind="ExternalInput").ap()
    dmask_in = nc.dram_tensor("dmask", [P, P], F32, kind="ExternalInput").ap()
    out = nc.dram_tensor("out", [S, QROWS], F32, kind="ExternalOutput").ap()

    with tile.TileContext(nc) as tc:
        with (
            tc.tile_pool(name="consts", bufs=1) as cp,
            tc.tile_pool(name="nat", bufs=2) as natp,
            tc.tile_pool(name="wT", bufs=1) as wtp,
            tc.tile_pool(name="xT", bufs=1) as xtp,
            tc.tile_pool(name="persist", bufs=1) as pp,
            tc.tile_pool(name="work", bufs=2) as wk,
            tc.tile_pool(name="dram", bufs=4, space="DRAM") as dp,
            tc.tile_pool(name="ps_mm", bufs=3, space="PSUM") as ps_mm,
            tc.tile_pool(name="ps_o", bufs=2, space="PSUM") as ps_o,
            tc.tile_pool(name="ps_tr", bufs=1, space="PSUM") as ps_tr,
            tc.tile_pool(name="ps_fp", bufs=1, space="PSUM") as ps_fp,
            tc.tile_pool(name="ps_ssq", bufs=1, space="PSUM") as ps_ssq,
        ):
            # ---- constants (loaded once; reused by every unrolled iteration) ----
            ident = cp.tile([P, P], F32R, tag="ident")
            nc.sync.dma_start(ident[:], ident_in[:].bitcast(F32R))
            ones = cp.tile([P, 1], F32R, tag="ones")
            nc.sync.dma_start(ones[:], ones_in[:].bitcast(F32R))
            onesrow = cp.tile([1, HD], F32R, tag="onesrow")
            nc.sync.dma_start(onesrow[:], onesrow_in[:].bitcast(F32R))
            bd = cp.tile([P, 2], F32R, tag="bd")
            nc.sync.dma_start(bd[:], bd_in[:].bitcast(F32R))
            bd2 = cp.tile([2, P], F32R, tag="bd2")
            nc.sync.dma_start(bd2[:], bd2_in[:].bitcast(F32R))
            dmask = cp.tile([P, P], F32R, tag="dmask")
            nc.sync.dma_start(dmask[:], dmask_in[:].bitcast(F32R))
            epsb = cp.tile([2, 1], F32, tag="epsb")
            nc.any.memset(epsb[:], EPS)
            g8 = cp.tile([2, 2], F32, tag="g8")
            nc.sync.dma_start(g8[:], qgain_in[:])
            nc.scalar.mul(g8[:], g8[:], 0.125)

            for rep in range(repeat):
                _iteration(
                    nc, tc, rep, phases, no_cc, groups, group_size,
                    xT_in, wq_in, wkv_in, wp_in, cos2_in, sin2_in, out,
                    ident, ones, onesrow, bd, bd2, dmask, epsb, g8,
                    natp, wtp, xtp, pp, wk, dp, ps_mm, ps_o, ps_tr, ps_fp, ps_ssq,
                )

    nc.compile()
    return nc


def _iteration(
    nc, tc, rep, phases, no_cc, groups, group_size,
    xT_in, wq_in, wkv_in, wp_in, cos2_in, sin2_in, out,
    ident, ones, onesrow, bd, bd2, dmask, epsb, g8,
    natp, wtp, xtp, pp, wk, dp, ps_mm, ps_o, ps_tr, ps_fp, ps_ssq,
):
    # ---- weights: int4 fake-quant + transpose ----
    # scale = max(rowmax(|w|), 1e-8)/7 computed exactly; n = round-half-even
    # (w * (1/scale)) via the 2^23 magic trick with a Newton-refined
    # reciprocal ([P,1] ops only); wdq = (n_magic - MAGIC) * scale fused in
    # one DVE pass. Boundary-flip probability per element is ~1e-7 — far
    # inside the error budget.
    wqT = wtp.tile([P, KT, QROWS], F32R, tag="wqT")
    wkvT = wtp.tile([P, KT, 2 * HD], F32R, tag="wkvT")
    wpT = wtp.tile([P, KT, QROWS], F32R, tag="wpT")
    for src, dstT, nblk in ((wq_in, wqT, 2), (wkv_in, wkvT, 1), (wp_in, wpT, 2)):
        pb = src.shape[0] // nblk
        for blk in range(nblk):
            w_nat = natp.tile([P, D], F32, tag="w_nat", bufs=2)
            nc.gpsimd.dma_start(w_nat[:pb], src[blk * pb : (blk + 1) * pb, :])
            m = wk.tile([P, 1], F32, tag="q_m", bufs=1)
            nc.vector.tensor_reduce(
                m[:pb], w_nat[:pb], axis=mybir.AxisListType.X, op=ALU.max,
                apply_absolute_value=True,
            )
            nc.vector.tensor_scalar(m[:pb], m[:pb], 1e-8, None, ALU.max)
            # scale = fl(mx/7) exactly: q0 = mx*C17; r = mx - 7*q0 computed
            # exactly as (mx - 8*q0) + q0 (8*q0 exact, both sums Sterbenz);
            # s = q0 + r*C17 is then the correctly rounded quotient.
            C17 = 0.14285714285714285
            scale = wk.tile([P, 1], F32, tag="q_scale", bufs=1)
            nc.vector.tensor_scalar(scale[:pb], m[:pb], C17, None, ALU.mult)
            tq = wk.tile([P, 1], F32, tag="q_tmp", bufs=1)
            nc.vector.tensor_scalar(tq[:pb], scale[:pb], -8.0, None, ALU.mult)
            nc.vector.tensor_tensor(tq[:pb], tq[:pb], m[:pb], ALU.add)
            nc.vector.tensor_tensor(tq[:pb], tq[:pb], scale[:pb], ALU.add)
            nc.vector.tensor_scalar(tq[:pb], tq[:pb], C17, None, ALU.mult)
            nc.vector.tensor_tensor(scale[:pb], scale[:pb], tq[:pb], ALU.add)
            rsc = wk.tile([P, 1], F32, tag="q_rsc", bufs=1)
            with nc.allow_low_precision(reason="quant reciprocal"):
                nc.vector.reciprocal(rsc[:pb], scale[:pb])
            # one Newton step: rsc *= (2 - scale*rsc)
            nt = wk.tile([P, 1], F32, tag="q_nt", bufs=1)
            nc.vector.tensor_scalar_mul(nt[:pb], rsc[:pb], scale[:pb])
            nc.vector.tensor_scalar(nt[:pb], nt[:pb], -1.0, 2.0, ALU.mult, ALU.add)
            nc.vector.tensor_scalar_mul(rsc[:pb], rsc[:pb], nt[:pb])
            # round pass (ACT): t = w*rsc + MAGIC; dequant (DVE, fused):
            # wdq = (t - MAGIC) * scale
            tmag = wk.tile([P, D], F32, tag="q_tmag", bufs=1)
            nc.scalar.activation(
                tmag[:pb], w_nat[:pb], AF.Copy, bias=MAGIC, scale=rsc[:pb]
            )
            wdq_t = wk.tile([P, D], F32R, tag="q_wdq", bufs=1)
            wdq = wdq_t[:pb]
            nc.vector.tensor_scalar(
                wdq, tmag[:pb], -MAGIC, scale[:pb], ALU.add, ALU.mult
            )
            for k0 in range(0, KT, 4):
                tp = ps_tr.tile([P, 4 * P], F32R, tag="tr")
                for q in range(4):
                    nc.tensor.transpose(
                        tp[:, q * P : q * P + pb],
                        wdq[:, (k0 + q) * P : (k0 + q + 1) * P],
                        ident[:pb, :pb],
                    )
                nc.vector.tensor_copy(
                    dstT[:, k0 : k0 + 4, blk * pb : (blk + 1) * pb],
                    tp[:].rearrange("p (a b) -> p a b", a=4)[:, :, :pb],
                )

    # ---- xT: direct tiled load of the host-transposed activations ----
    xT = [
        xtp.tile([P, S], F32R, tag=f"xT{k}", name=f"r{rep}_xT{k}") for k in range(KT)
    ]
    for k in range(KT):
        nc.sync.dma_start(
            xT[k][:], xT_in[k * P : (k + 1) * P, :].bitcast(F32R)
        )

    # ---- persistent attention operands ----
    qTr = [
        pp.tile([HD, S], F32R, tag=f"qTr{h}", name=f"r{rep}_qTr{h}") for h in range(4)
    ]
    kTr = pp.tile([HD, S], F32R, tag="kTr")
    vAug = pp.tile([P, S // P, HD + 1], F32R, tag="vAug")

    def rope_and_scale(raw, fb_ps, cos_t, sin_t, rows, outs):
        """raw [rows, CH] f32 (pre-norm, pre-rope); fb_ps: psum rms*gain
        factor [rows, CH]; outs = [(dst f32r [64, CH], src row)] splits."""
        qsw = wk.tile([P, CH], F32, tag="qsw", bufs=1)
        for r0 in range(0, rows, HD):
            nc.gpsimd.tensor_copy(qsw[r0 : r0 + 32], raw[r0 + 32 : r0 + 64])
            nc.gpsimd.tensor_copy(qsw[r0 + 32 : r0 + 64], raw[r0 : r0 + 32])
        t2 = wk.tile([P, CH], F32, tag="t2", bufs=1)
        nc.vector.tensor_mul(t2[:rows], raw[:], cos_t[:rows])
        nc.vector.tensor_mul(qsw[:rows], qsw[:rows], sin_t[:rows])
        nc.vector.tensor_add(qsw[:rows], qsw[:rows], t2[:rows])
        for dst, lo in outs:
            nc.vector.tensor_mul(dst, qsw[lo : lo + HD], fb_ps[lo : lo + HD])

    # ---- projections (per chunk) ----
    for c in range(NCH if phases >= 1 else 0):
        sl = slice(c * CH, (c + 1) * CH)
        cos_t = wk.tile([P, CH], F32, tag="cos_t", bufs=1)
        nc.gpsimd.dma_start(cos_t[:], cos2_in[:, sl])
        sin_t = wk.tile([P, CH], F32, tag="sin_t", bufs=1)
        nc.gpsimd.dma_start(sin_t[:], sin2_in[:, sl])

        # q: two head pairs
        for mblk in range(2):
            pq = ps_mm.tile([P, CH], F32, tag="mm")
            for k in range(KT):
                nc.tensor.matmul(
                    pq[:],
                    wqT[:, k, mblk * P : (mblk + 1) * P],
                    xT[k][:, sl],
                    start=(k == 0),
                    stop=(k == KT - 1),
                )
            q_raw = wk.tile([P, CH], F32, tag="raw", bufs=1)
            nc.scalar.copy(q_raw[:], pq[:])
            q2 = wk.tile([P, CH], F32R, tag="sq", bufs=1)
            nc.scalar.activation(q2[:], pq[:], AF.Square)
            ssq = ps_ssq.tile([2, CH], F32, tag="ssq")
            nc.tensor.matmul(ssq[:], bd[:, :], q2[:], start=True, stop=True)
            srms = wk.tile([2, CH], F32, tag="srms", bufs=1)
            nc.scalar.activation(
                srms[:], ssq[:], AF.Sqrt, bias=epsb[:], scale=1.0 / HD
            )
            rfac = wk.tile([2, CH], F32R, tag="rfac", bufs=1)
            with nc.allow_low_precision(reason="f32r matmul feed"):
                nc.vector.reciprocal(rfac[:], srms[:])
            nc.vector.tensor_scalar_mul(rfac[:], rfac[:], g8[0:2, mblk : mblk + 1])
            fb = ps_mm.tile([P, CH], F32, tag="mm")
            nc.tensor.matmul(fb[:], bd2[:], rfac[:], start=True, stop=True)
            rope_and_scale(
                q_raw[:], fb, cos_t, sin_t, P,
                [(qTr[2 * mblk][:, sl], 0), (qTr[2 * mblk + 1][:, sl], HD)],
            )

        # kv
        pkv = ps_mm.tile([P, CH], F32, tag="mm")
        for k in range(KT):
            nc.tensor.matmul(
                pkv[:], wkvT[:, k, :], xT[k][:, sl],
                start=(k == 0), stop=(k == KT - 1),
            )
        kv_raw = wk.tile([P, CH], F32, tag="raw", bufs=1)
        nc.scalar.copy(kv_raw[:], pkv[:])
        k2 = wk.tile([P, CH], F32R, tag="sq", bufs=1)
        nc.scalar.activation(k2[:HD], pkv[:HD], AF.Square)
        ssk = ps_ssq.tile([2, CH], F32, tag="ssq")
        nc.tensor.matmul(ssk[0:1], ones[:HD], k2[:HD], start=True, stop=True)
        srk = wk.tile([2, CH], F32, tag="srms", bufs=1)
        nc.scalar.activation(
            srk[0:1], ssk[0:1], AF.Sqrt, bias=epsb[0:1], scale=1.0 / HD
        )
        rfk = wk.tile([2, CH], F32R, tag="rfac", bufs=1)
        with nc.allow_low_precision(reason="f32r matmul feed"):
            nc.vector.reciprocal(rfk[0:1], srk[0:1])
        fbk = ps_mm.tile([P, CH], F32, tag="mm")
        nc.tensor.matmul(fbk[:HD], onesrow[:], rfk[0:1], start=True, stop=True)
        rope_and_scale(kv_raw[:HD], fbk, cos_t, sin_t, HD, [(kTr[:, sl], 0)])
        # v half -> vAug tiles (s on partitions) + ones column
        v_r = wk.tile([P, CH], F32R, tag="v_r", bufs=1)
        nc.scalar.copy(v_r[:HD], kv_raw[HD:])
        tpv = ps_tr.tile([P, 4 * P], F32R, tag="tr")
        for st in range(CH // P):
            nc.tensor.transpose(
                tpv[:, st * P : st * P + HD],
                v_r[:HD, st * P : (st + 1) * P],
                ident[:HD, :HD],
            )
        j0 = c * (CH // P)
        nc.vector.tensor_copy(
            vAug[:, j0 : j0 + 4, 0:HD],
            tpv[:].rearrange("p (a b) -> p a b", a=4)[:, :, :HD],
        )
        nc.vector.tensor_copy(
            vAug[:, j0 : j0 + 4, HD : HD + 1],
            ones[:, 0:1, None].to_broadcast((P, 4, 1)),
        )

    # ---- attention + collective + output projection, per chunk ----
    for c in range(NCH if phases >= 2 else 0):
        cc_in = dp.tile([QROWS, CH], F32, tag="cc_in")
        cc_out = dp.tile([G * QROWS, CH], F32, tag="cc_out")
        for h in range(4):
            po = ps_o.tile([HD + 1, CH], F32, tag="po")
            njc = 4 * c + 4
            for j in range(njc):
                r = j - 4 * c  # >= 0 only on causal-boundary tiles
                f0 = r * P if r > 0 else 0
                psc = ps_mm.tile([P, CH], F32, tag="mm")
                nc.tensor.matmul(
                    psc[:, f0:],
                    kTr[:, j * P : (j + 1) * P],
                    qTr[h][:, c * CH + f0 : (c + 1) * CH],
                    start=True,
                    stop=True,
                    skip_group_check=True,
                )
                et = wk.tile([P, CH], F32R, tag="et", bufs=2)
                nc.scalar.activation(et[:, f0:], psc[:, f0:], AF.Exp)
                if r >= 0:
                    nc.vector.tensor_mul(
                        et[:, r * P : (r + 1) * P],
                        et[:, r * P : (r + 1) * P],
                        dmask[:],
                    )
                nc.tensor.matmul(
                    po[:, f0:],
                    vAug[:, j, :],
                    et[:, f0:],
                    start=(j == 0),
                    stop=(j == njc - 1),
                    skip_group_check=True,
                )
            # divide by the softmax sum (row HD of po), broadcast via PE
            rs = wk.tile([1, CH], F32R, tag="rs", bufs=1)
            with nc.allow_low_precision(reason="f32r matmul feed"):
                nc.vector.reciprocal(rs[:], po[HD : HD + 1, :])
            pr = ps_mm.tile([P, CH], F32, tag="mm")
            nc.tensor.matmul(pr[:HD], onesrow[:], rs[:], start=True, stop=True)
            rb = wk.tile([HD, CH], F32, tag="rb", bufs=1)
            nc.vector.tensor_copy(rb[:], pr[:HD])
            yt = wk.tile([HD, CH], F32R, tag="yt", bufs=1)
            nc.vector.tensor_mul(yt[:], po[:HD, :], rb[:])
            nc.sync.dma_start(cc_in[h * HD : (h + 1) * HD, :], yt[:].bitcast(F32))
        if no_cc:
            for gg in range(group_size):
                nc.sync.dma_start(cc_out[gg * QROWS : (gg + 1) * QROWS, :], cc_in[:])
        else:
            nc.gpsimd.collective_compute(
                "AllGather",
                ALU.bypass,
                replica_groups=groups,
                ins=[cc_in.opt()],
                outs=[cc_out.opt()],
            )
        # output projection for this chunk: out[sl] = y_chunk @ wp_g.T
        ya = [
            wk.tile([P, CH], F32R, tag="ya", bufs=8, name=f"r{rep}_ya{c}_{k}")
            for k in range(KT)
        ]
        for k in range(KT):
            nc.sync.dma_start(ya[k][:], cc_out[k * P : (k + 1) * P, :].bitcast(F32R))
        for sh in range(4):
            pf = ps_fp.tile([P, QROWS], F32, tag="fp")
            for k in range(KT):
                nc.tensor.matmul(
                    pf[:], ya[k][:, sh * P : (sh + 1) * P], wpT[:, k, :],
                    start=(k == 0), stop=(k == KT - 1),
                )
            ot = wk.tile([P, QROWS], F32, tag="ot", bufs=1)
            nc.scalar.copy(ot[:], pf[:])
            nc.gpsimd.dma_start(
                out[c * CH + sh * P : c * CH + (sh + 1) * P, :], ot[:]
            )


@functools.lru_cache(maxsize=None)
def get_nc():
    return build_nc()


@functools.lru_cache(maxsize=None)
def host_consts():
    inv_freq = (
        1.0 / (ROPE_BASE ** (np.arange(0, HD, 2, dtype=np.float32) / HD))
    ).astype(np.float32)
    freqs = np.outer(np.arange(S, dtype=np.float32), inv_freq)  # [S, 32]
    cosT = np.cos(freqs).T.astype(np.float32)  # [32, S]
    sinT = np.sin(freqs).T.astype(np.float32)
    cos2 = np.ascontiguousarray(np.tile(cosT, (4, 1)))  # [128, S]
    sin2 = np.ascontiguousarray(
        np.concatenate([sinT, -sinT, sinT, -sinT], axis=0)
    ).astype(np.float32)
    ident = np.eye(P, dtype=np.float32)
    ones = np.ones((P, 1), np.float32)
    onesrow = np.ones((1, HD), np.float32)
    bd = np.zeros((P, 2), np.float32)
    bd[0:HD, 0] = 1.0
    bd[HD:P, 1] = 1.0
    bd2 = np.ascontiguousarray(bd.T)
    # dmask[p, u] = 1 if u >= p (valid region of the causal diagonal tile)
    dmask = (np.arange(P)[None, :] >= np.arange(P)[:, None]).astype(np.float32)
    return dict(
        cos2=cos2, sin2=sin2, ident=ident, ones=ones, onesrow=onesrow,
        bd=bd, bd2=bd2, dmask=dmask,
    )


def make_in_maps(x, w_q, w_k, w_v, w_proj, q_gain, n_cores=N_CORES, group_size=G):
    consts = host_consts()
    xT_b = [np.ascontiguousarray(np.asarray(x[b]).T) for b in range(B)]
    in_maps = []
    for core in range(n_cores):
        b, g = core // group_size, core % group_size
        wkv = np.concatenate(
            [w_k[g * HD : (g + 1) * HD, :], w_v[g * HD : (g + 1) * HD, :]], axis=0
        )
        in_maps.append(
            dict(
                xT=xT_b[b],
                wq=np.ascontiguousarray(w_q[g * QROWS : (g + 1) * QROWS, :]),
                wkv=np.ascontiguousarray(wkv),
                wp=np.ascontiguousarray(w_proj[g * QROWS : (g + 1) * QROWS, :]),
                qgain=np.ascontiguousarray(q_gain[4 * g : 4 * g + 4].reshape(2, 2).T),
                **consts,
            )
        )
    return in_maps


def assemble(results, n_cores=N_CORES, group_size=G):
    out = np.empty((B, S, D), np.float32)
    for core in range(n_cores):
        b, g = core // group_size, core % group_size
        out[b, :, g * QROWS : (g + 1) * QROWS] = results[core]["out"]
    return out


def kernel(**inputs):
    x = np.asarray(inputs["x"], np.float32)
    w_q = np.asarray(inputs["w_q"], np.float32)
    w_k = np.asarray(inputs["w_k"], np.float32)
    w_v = np.asarray(inputs["w_v"], np.float32)
    w_proj = np.asarray(inputs["w_proj"], np.float32)
    q_gain = np.asarray(inputs["q_gain"], np.float32)

    nc = get_nc()
    in_maps = make_in_maps(x, w_q, w_k, w_v, w_proj, q_gain)
    res = run_bass_kernel_spmd(nc, in_maps, list(range(N_CORES)))
    return assemble(res.results)
